# revision 1
# baseline (speedup 1.0000x reference)
"""Fused multi-head attention (QKV + RoPE2D + softmax + out-proj) on 8 TRN2 cores.

Sharding: batch-parallel. B == n_cores == 8, so each core runs one batch
element end-to-end; weights are replicated. No collectives needed.

Per-core dataflow (matmul operands in KDTYPE, accumulation in f32 PSUM):
  phase 1: v[n,dd]   = x @ w_v.T         (lhsT = xT tiles, rhs = w_vT tiles)
           qkT[D,n]  = (x @ w_{q,k}.T).T (lhsT = w chunks,  rhs = xT tiles)
           RoPE on qkT: rot = PERM @ qkT (matmul), then
           qk' = qkT*cos2 + rot*sin2   (signs folded into sin2 host-side)
  phase 2: per head h:
           sT[j,i] = k'_h-tiles.T @ q'_h  (K=64)
           e = exp(sT/8) on ACT, PSUM->SBUF
           av = [v_h | ones].T @ e accumulated over j-tiles ->
                rows 0:64 = unnormalized out.T, row 64 = softmax sums
           outT rows for h = av[0:64]; sums collected, then per 8-head batch:
           reciprocal + K=8 selection-matmul broadcast + multiply
  phase 3: y[n,e] = outT-tiles.T @ w_projT + bias -> DMA out

The next pair's QKV/RoPE matmuls are interleaved into the current pair's
attention emission (generator-based software pipelining) so the in-order
PE queue always has ready work during exp waits.
"""

import os
import numpy as np

B, N, C = 8, 1024, 1024
H, HD = 16, 64
P = 128
NT = N // P          # 8 n-tiles
CT = C // P          # 8 c-tiles
TP = H // 2          # 8 head-pairs (qk D-tiles per q/k)
SCALE = HD ** -0.5   # 1/8

KDTYPE = os.environ.get("BASS_ATTN_DTYPE", "bfloat16")

_CACHE = {}


def _build_nc():
    import concourse.mybir as mybir
    from concourse import bacc, tile
    from contextlib import ExitStack

    f32 = mybir.dt.float32
    mdt = getattr(mybir.dt, KDTYPE)

    nc = bacc.Bacc(
        "TRN2", target_bir_lowering=False, debug=False,
        enable_asserts=False, num_devices=B,
    )

    xT_d = nc.dram_tensor("xT", [C, N], mdt, kind="ExternalInput")
    cos2_d = nc.dram_tensor("cos2", [P, N], f32, kind="ExternalInput")
    sin2_d = nc.dram_tensor("sin2", [P, N], f32, kind="ExternalInput")
    perm_d = nc.dram_tensor("perm", [P, P], mdt, kind="ExternalInput")
    wq_d = nc.dram_tensor("wq", [TP, CT, P, P], mdt, kind="ExternalInput")
    wk_d = nc.dram_tensor("wk", [TP, CT, P, P], mdt, kind="ExternalInput")
    wv_d = nc.dram_tensor("wv", [CT, P, C], mdt, kind="ExternalInput")
    wp_d = nc.dram_tensor("wp", [CT, P, C], mdt, kind="ExternalInput")
    sel_d = nc.dram_tensor("sel", [TP, TP, P], mdt, kind="ExternalInput")
    bias_d = nc.dram_tensor("bias", [1, C], f32, kind="ExternalInput")
    out_d = nc.dram_tensor("out", [N, C], f32, kind="ExternalOutput")

    with tile.TileContext(nc) as tc, ExitStack() as ctx:
        const = ctx.enter_context(tc.tile_pool(name="const", bufs=1))
        vpool = ctx.enter_context(tc.tile_pool(name="vpool", bufs=1))
        otpool = ctx.enter_context(tc.tile_pool(name="otpool", bufs=1))
        qkpre = ctx.enter_context(tc.tile_pool(name="qkpre", bufs=2))
        qkfin = ctx.enter_context(tc.tile_pool(name="qkfin", bufs=6))
        expp = ctx.enter_context(tc.tile_pool(name="expp", bufs=3))
        rcpp = ctx.enter_context(tc.tile_pool(name="rcpp", bufs=2))
        tmpp = ctx.enter_context(tc.tile_pool(name="tmpp", bufs=2))
        sT_ps = ctx.enter_context(tc.tile_pool(name="sT_ps", bufs=2, space="PSUM"))
        av_ps = ctx.enter_context(tc.tile_pool(name="av_ps", bufs=1, space="PSUM"))
        mi_ps = ctx.enter_context(tc.tile_pool(name="mi_ps", bufs=2, space="PSUM"))

        # constants
        perm = const.tile([P, P], mdt)
        nc.sync.dma_start(perm[:], perm_d[:])
        cos2 = const.tile([P, N], f32)
        nc.sync.dma_start(cos2[:], cos2_d[:])
        sin2 = const.tile([P, N], f32)
        nc.sync.dma_start(sin2[:], sin2_d[:])
        bias_bc = const.tile([P, C], f32)
        nc.sync.dma_start(bias_bc[:1, :], bias_d[:])
        nc.gpsimd.partition_broadcast(bias_bc[:], bias_bc[:1, :])

        # v storage: [128 j-local, NT j-tiles, H heads x (64 v + 1 ones col)]
        v_all = vpool.tile([P, NT, H * (HD + 1)], mdt)
        ones_c = const.tile([P, H], f32)
        nc.vector.memset(ones_c[:], 1.0)
        for jt in range(NT):
            nc.vector.tensor_copy(
                v_all[:, jt, :].rearrange("p (h c) -> p h c", c=HD + 1)[:, :, HD:],
                ones_c[:].rearrange("p (h o) -> p h o", o=1))
        # out.T accumulator: [128 c-local, CT c-tiles, 1024 n]
        outT = otpool.tile([P, CT, N], mdt)
        # per-head softmax sums: two batches of 8 heads so the first
        # batch's normalization overlaps the second batch's attention
        sums_b = [otpool.tile([TP, N], f32, tag=f"sums{i}", name=f"sums{i}")
                  for i in range(2)]
        rcp_b = [otpool.tile([TP, N], f32, tag=f"rcpf{i}", name=f"rcpf{i}")
                 for i in range(2)]
        rcp16_b = [otpool.tile([TP, N], mdt, tag=f"rcp16{i}", name=f"rcp16{i}")
                   for i in range(2)]
        selc = const.tile([TP, TP, P], mdt)  # head-pair selection matrices
        nc.sync.dma_start(selc[:], sel_d[:].rearrange("a h p -> h a p"))

        with tc.tile_pool(name="xtp", bufs=1) as xtp, \
             tc.tile_pool(name="wvp", bufs=1) as wvp, \
             tc.tile_pool(name="wch", bufs=3) as wch:
            xt = xtp.tile([P, CT, N], mdt)       # xT tiles, ct-indexed
            wv = wvp.tile([P, CT, C], mdt)
            # split column-wise and interleave so the first v matmuls (which
            # touch only the leading columns) start as early as possible
            for cs in (slice(0, 512), slice(512, N)):
                for ct in range(CT):
                    nc.sync.dma_start(xt[:, ct, cs],
                                      xT_d[ct * P:(ct + 1) * P, cs])
                    nc.sync.dma_start(wv[:, ct, cs], wv_d[ct][:, cs])

            # ---- phase 1a: v = x @ w_v.T (straight orientation) ----
            for nt in range(NT):
                for ch in range(2):
                    vps = mi_ps.tile([P, 512], f32, tag="mi", name="vps")
                    for ct in range(CT):
                        nc.tensor.matmul(
                            vps[:],
                            xt[:, ct, nt * P:(nt + 1) * P],
                            wv[:, ct, ch * 512:(ch + 1) * 512],
                            start=(ct == 0), stop=(ct == CT - 1),
                        )
                    nc.vector.tensor_copy(
                        v_all[:, nt, :].rearrange(
                            "p (h c) -> p h c", c=HD + 1)[:, 8 * ch:8 * ch + 8, :HD],
                        vps[:])

            # ---- phase 1b+2: software-pipelined head-pairs ----
            qk_fin = {}

            def prepare_pair(t):
                """Generator: qkv D-tiles + RoPE for pair t, yielding after
                each PE instruction so it can interleave into attention."""
                qk_tiles = []
                for wsrc in (wq_d, wk_d):
                    pre = qkpre.tile([P, N], mdt, tag="pre", name="pre")
                    wcht = wch.tile([P, CT, P], mdt, tag="w", name="wcht")
                    nc.sync.dma_start(
                        wcht[:], wsrc[t].rearrange("a p c -> p a c"))
                    for ch in range(2):
                        qps = mi_ps.tile([P, 512], f32, tag="mi", name="qps")
                        for ct in range(CT):
                            nc.tensor.matmul(
                                qps[:],
                                wcht[:, ct, :],
                                xt[:, ct, ch * 512:(ch + 1) * 512],
                                start=(ct == 0), stop=(ct == CT - 1),
                            )
                            yield
                        nc.vector.tensor_copy(
                            pre[:, ch * 512:(ch + 1) * 512], qps[:])
                    # RoPE: fin = pre*cos2 + (PERM @ pre)*sin2
                    fin = qkfin.tile([P, N], mdt, tag="fin", name="fin")
                    for ch in range(2):
                        sl = slice(ch * 512, (ch + 1) * 512)
                        rot = mi_ps.tile([P, 512], f32, tag="mi", name="rot")
                        nc.tensor.matmul(rot[:], perm[:], pre[:, sl],
                                         start=True, stop=True)
                        yield
                        tmp = tmpp.tile([P, 512], f32, tag="tmp", name="tmp")
                        nc.vector.tensor_mul(tmp[:], pre[:, sl], cos2[:, sl])
                        nc.vector.tensor_mul(fin[:, sl], rot[:], sin2[:, sl])
                        nc.vector.tensor_add(fin[:, sl], fin[:, sl], tmp[:])
                    qk_tiles.append(fin)
                qk_fin[t] = qk_tiles

            def pull(feeder, k):
                if feeder is None:
                    return None
                for _ in range(k):
                    if next(feeder, "done") == "done":
                        return None
                return feeder

            feeder = prepare_pair(0)
            pull(feeder, 10 ** 6)
            for t in range(TP):
                feeder = prepare_pair(t + 1) if t + 1 < TP else None
                qf, kf = qk_fin.pop(t)

                for hh in range(2):  # head = 2*t + hh
                    h = 2 * t + hh
                    ro = slice(64 * hh, 64 * hh + 64)
                    av = av_ps.tile([HD + 1, N], f32, tag="av", name="av")
                    for jt in range(NT):
                        sT = sT_ps.tile([P, N], f32, tag="sT", name="sT")
                        for ch in range(2):
                            nc.tensor.matmul(
                                sT[:, ch * 512:(ch + 1) * 512],
                                kf[ro, jt * P:(jt + 1) * P],
                                qf[ro, ch * 512:(ch + 1) * 512],
                                start=True, stop=True,
                            )
                        ex = expp.tile([P, N], mdt, tag="ex", name="ex")
                        nc.scalar.activation(
                            ex[:], sT[:],
                            mybir.ActivationFunctionType.Exp, scale=SCALE)
                        # stationary [K=128 j, M=65]: head h's v cols + ones
                        vh = v_all[:, jt, h * (HD + 1):(h + 1) * (HD + 1)]
                        for ch in range(2):
                            nc.tensor.matmul(
                                av[:, ch * 512:(ch + 1) * 512],
                                vh,
                                ex[:, ch * 512:(ch + 1) * 512],
                                start=(jt == 0), stop=(jt == NT - 1),
                            )
                        feeder = pull(feeder, 3)
                    # av rows 0:64 = unnormalized out.T, row 64 = softmax sums
                    nc.vector.tensor_copy(outT[ro, t, :], av[:HD, :])
                    # DVE outputs must start at partition 0; bounce the sums
                    # row through SBUF and DMA it onto partition h%8
                    cp1 = rcpp.tile([1, N], f32, tag="cp1", name="cp1")
                    nc.vector.tensor_copy(cp1[:], av[HD:HD + 1, :])
                    nc.sync.dma_start(sums_b[h // 8][h % 8:h % 8 + 1, :], cp1[:])
                pull(feeder, 10 ** 6)

                if t == 5 or t == TP - 1:
                    # normalize a batch of 8 heads: K=8 selection matmul
                    # broadcasts each head's reciprocal sum over its 64 outT
                    # rows; batch 0 is emitted two pairs late so its chain is
                    # off the critical path
                    g = 0 if t == 5 else 1
                    nc.vector.reciprocal_approx_fast(rcp_b[g][:], sums_b[g][:])
                    nc.vector.tensor_copy(rcp16_b[g][:], rcp_b[g][:])
                    for tt in range(4 * g, 4 * g + 4):
                        for ch in range(2):
                            sl = slice(ch * 512, (ch + 1) * 512)
                            rb = mi_ps.tile([P, 512], f32, tag="mi", name="rb")
                            nc.tensor.matmul(rb[:], selc[:, tt, :],
                                             rcp16_b[g][:, sl],
                                             start=True, stop=True)
                            nc.vector.tensor_mul(outT[:HD, tt, sl],
                                                 outT[:HD, tt, sl], rb[:HD, :])
                            nc.vector.tensor_mul(outT[HD:, tt, sl],
                                                 outT[HD:, tt, sl], rb[HD:, :])

        # ---- phase 3: y = outT.T @ w_projT + bias ----
        with tc.tile_pool(name="wpp", bufs=1) as wpp, \
             tc.tile_pool(name="ybp", bufs=3) as ybp:
            wp = wpp.tile([P, CT, C], mdt)
            for ct in range(CT):
                nc.sync.dma_start(wp[:, ct, :], wp_d[ct])
            for nt in range(NT):
                yps = sT_ps.tile([P, N], f32, tag="sT", name="yps")
                for ch in range(2):
                    sl = slice(ch * 512, (ch + 1) * 512)
                    for ct in range(CT):
                        nc.tensor.matmul(
                            yps[:, sl],
                            outT[:, ct, nt * P:(nt + 1) * P],
                            wp[:, ct, sl],
                            start=(ct == 0), stop=(ct == CT - 1),
                        )
                yb = ybp.tile([P, N], f32, tag="yb", name="yb")
                nc.vector.tensor_add(yb[:], yps[:], bias_bc[:])
                nc.sync.dma_start(out_d[nt * P:(nt + 1) * P, :], yb[:])

    nc.compile()
    return nc


def get_nc():
    if "nc" not in _CACHE:
        _CACHE["nc"] = _build_nc()
    return _CACHE["nc"]


def _host_inputs(x, xpos, w_qkv, w_proj, b_proj):
    """Host-side reshapes: transposes, RoPE tables, weight packing."""
    x = np.asarray(x, dtype=np.float32)
    xpos = np.asarray(xpos)
    w_qkv = np.asarray(w_qkv, dtype=np.float32)
    w_proj = np.asarray(w_proj, dtype=np.float32)
    b_proj = np.asarray(b_proj, dtype=np.float32).reshape(1, C)

    xT = np.ascontiguousarray(x.transpose(0, 2, 1))  # [B, C, N]

    # RoPE tables in [d, n] orientation, two head-copies stacked to 128 rows.
    inv_freq = (100.0 ** (-np.arange(16, dtype=np.float64) / 16.0))
    py = xpos[..., 0].astype(np.float64)  # [B, N]
    px = xpos[..., 1].astype(np.float64)
    angy = py[:, :, None] * inv_freq      # [B, N, 16]
    angx = px[:, :, None] * inv_freq
    cos64 = np.concatenate(
        [np.cos(angy), np.cos(angy), np.cos(angx), np.cos(angx)], axis=2)
    sin64 = np.concatenate(
        [-np.sin(angy), np.sin(angy), -np.sin(angx), np.sin(angx)], axis=2)
    cos2 = np.ascontiguousarray(
        np.tile(cos64, (1, 1, 2)).transpose(0, 2, 1)).astype(np.float32)
    sin2 = np.ascontiguousarray(
        np.tile(sin64, (1, 1, 2)).transpose(0, 2, 1)).astype(np.float32)

    # permutation matrix: sigma(d) = d XOR 16 within each 64-block
    r = np.arange(P)
    sig = (r // 64) * 64 + ((r % 64) ^ 16)
    perm = np.zeros((P, P), dtype=np.float32)
    perm[sig, r] = 1.0  # perm[k, m] = 1 iff k == sigma(m)

    wq = np.zeros((TP, CT, P, P), dtype=np.float32)
    wk = np.zeros((TP, CT, P, P), dtype=np.float32)
    for t in range(TP):
        for ct in range(CT):
            wq[t, ct] = w_qkv[t * P:(t + 1) * P, ct * P:(ct + 1) * P].T
            wk[t, ct] = w_qkv[C + t * P:C + (t + 1) * P, ct * P:(ct + 1) * P].T
    wv = np.ascontiguousarray(
        w_qkv[2 * C:3 * C, :].T.reshape(CT, P, C))   # [ct][c-local, dd]
    wp = np.ascontiguousarray(w_proj.T.reshape(CT, P, C))  # [ct][c-local, e]

    sel = np.zeros((TP, TP, P), dtype=np.float32)
    for t in range(TP):
        sel[t, 2 * (t % 4), :HD] = 1.0
        sel[t, 2 * (t % 4) + 1, HD:] = 1.0

    if KDTYPE == "bfloat16":
        import ml_dtypes

        def mcast(a):
            return np.ascontiguousarray(a).astype(ml_dtypes.bfloat16)
    else:
        def mcast(a):
            return np.ascontiguousarray(a)

    shared = dict(perm=mcast(perm), wq=mcast(wq), wk=mcast(wk),
                  wv=mcast(wv), wp=mcast(wp), sel=mcast(sel), bias=b_proj)
    in_maps = []
    for b in range(B):
        m = dict(shared)
        m["xT"] = mcast(xT[b])
        m["cos2"] = cos2[b]
        m["sin2"] = sin2[b]
        in_maps.append(m)
    return in_maps


def kernel(x, xpos, w_qkv, w_proj, b_proj):
    from concourse import bass_utils

    nc = get_nc()
    in_maps = _host_inputs(x, xpos, w_qkv, w_proj, b_proj)
    res = bass_utils.run_bass_kernel_spmd(
        nc, in_maps, core_ids=list(range(B)),
        trace=bool(int(os.environ.get("BASS_ATTN_TRACE", "0"))),
    )
    out = np.stack([res.results[b]["out"] for b in range(B)], axis=0)
    _CACHE["last_results"] = res
    return out



# revision 11
# speedup vs baseline: 1.1143x; 1.1143x over previous
"""Fused multi-head attention (QKV + RoPE2D + softmax + out-proj) on 8 TRN2 cores.

Sharding: batch-parallel. B == n_cores == 8, so each core runs one batch
element end-to-end; weights are replicated. No collectives needed.

Per-core dataflow (matmul operands in bf16, accumulation in f32 PSUM):
  qkT[D,n] = (x @ w_{q,k}.T).T   (lhsT = w chunks, rhs = xT tiles)
  RoPE on qkT: rot = stream_shuffle(qkT) (lane permute i^16 per 32-block),
  qk' = qkT*cos2 + rot*sin2      (signs folded into sin2 host-side; all bf16)
  v[n,dd] = x @ w_v.T            (v-proj groups streamed inside the feeder)
  attention per head-PAIR t (heads A=2t on partitions 0:64 of qk', B=2t+1
  on 64:128): per (ch, jt):
     sT[:, 0:512]   = kA-tile.T @ qA-chunk   (K=64, PE rows 0-63)
     sT[:, 512:1024]= kB-tile.T @ qB-chunk   (K=64, PE rows 64-127)
     -> issued back-to-back, the two row-tiles stream CONCURRENTLY
     ex = exp(sT/8) on ACT, one [128,1024] call for both heads
     av_X += [v_X | ones].T @ ex[:, X-half]  (K=128, accumulated over jt,
              software-pipelined one step behind exp so PE never waits)
  softmax sums ride in av row 64; per-pair normalization: ACT copies the
  sums rows out of PSUM, DVE reciprocal, GPSIMD partition-broadcasts the
  per-head scales to 64 rows, one DVE multiply normalizes outT[:, t, :].
  y = outT.T @ w_projT (+bias via DVE add) -> DMA out; ct accumulation in
  pair-completion order so only pair 7's normalization is on the tail.

The next pair's QKV matmuls and the v-projection groups are interleaved
into the current pair's attention (generator-based software pipelining)
so the in-order PE queue always has ready work during exp waits.
"""

import os
import numpy as np

B, N, C = 8, 1024, 1024
H, HD = 16, 64
P = 128
NT = N // P          # 8 n-tiles
CT = C // P          # 8 c-tiles
TP = H // 2          # 8 head-pairs (qk D-tiles per q/k)
SCALE = HD ** -0.5   # 1/8

_CACHE = {}

SHUF_MASK = [i ^ 16 for i in range(32)]  # rotate_half as 32-lane permute


def _build_nc():
    import concourse.mybir as mybir
    from concourse import bacc, tile
    from contextlib import ExitStack

    f32 = mybir.dt.float32
    bf16 = mybir.dt.bfloat16
    EXP = mybir.ActivationFunctionType.Exp
    CPY = mybir.ActivationFunctionType.Copy

    nc = bacc.Bacc(
        "TRN2", target_bir_lowering=False, debug=False,
        enable_asserts=False, num_devices=B,
    )

    xT_d = nc.dram_tensor("xT", [C, N], bf16, kind="ExternalInput")
    cos2_d = nc.dram_tensor("cos2", [P, N], bf16, kind="ExternalInput")
    sin2_d = nc.dram_tensor("sin2", [P, N], bf16, kind="ExternalInput")
    wq_d = nc.dram_tensor("wq", [TP, CT, P, P], bf16, kind="ExternalInput")
    wk_d = nc.dram_tensor("wk", [TP, CT, P, P], bf16, kind="ExternalInput")
    wv_d = nc.dram_tensor("wv", [CT, P, C], bf16, kind="ExternalInput")
    wp_d = nc.dram_tensor("wp", [CT, P, C], bf16, kind="ExternalInput")
    sel2_d = nc.dram_tensor("sel2", [2, P], bf16, kind="ExternalInput")
    bias_d = nc.dram_tensor("bias", [1, C], f32, kind="ExternalInput")
    out_d = nc.dram_tensor("out", [N, C], f32, kind="ExternalOutput")

    with tile.TileContext(nc) as tc, ExitStack() as ctx:
        const = ctx.enter_context(tc.tile_pool(name="const", bufs=1))
        vpool = ctx.enter_context(tc.tile_pool(name="vpool", bufs=1))
        otpool = ctx.enter_context(tc.tile_pool(name="otpool", bufs=1))
        qkpre = ctx.enter_context(tc.tile_pool(name="qkpre", bufs=2))
        qkfin = ctx.enter_context(tc.tile_pool(name="qkfin", bufs=4))
        expp = ctx.enter_context(tc.tile_pool(name="expp", bufs=3))
        cpsp = ctx.enter_context(tc.tile_pool(name="cpsp", bufs=4))
        nrmp = ctx.enter_context(tc.tile_pool(name="nrmp", bufs=2))
        ybp = ctx.enter_context(tc.tile_pool(name="ybp", bufs=3))
        sT_ps = ctx.enter_context(tc.tile_pool(name="sT_ps", bufs=2, space="PSUM"))
        av_ps = ctx.enter_context(tc.tile_pool(name="av_ps", bufs=2, space="PSUM"))
        mi_ps = ctx.enter_context(tc.tile_pool(name="mi_ps", bufs=2, space="PSUM"))

        # ---- input DMA, priority order ----
        wqk = const.tile([P, TP, 2, CT, P], bf16)  # all q,k weight D-tiles
        for s, wsrc in ((0, wq_d), (1, wk_d)):
            nc.sync.dma_start(wqk[:, 0, s], wsrc[0].rearrange("a p c -> p a c"))
        xt = const.tile([P, CT, N], bf16)          # xT tiles, ct-indexed
        for ct in range(CT):
            nc.sync.dma_start(xt[:, ct, 0:512], xT_d[ct * P:(ct + 1) * P, 0:512])
        cos2 = const.tile([P, N], bf16)
        nc.sync.dma_start(cos2[:], cos2_d[:])
        sin2 = const.tile([P, N], bf16)
        nc.sync.dma_start(sin2[:], sin2_d[:])
        wv = const.tile([P, CT, C], bf16)
        for cs in (slice(0, 512), slice(512, C)):
            for ct in range(CT):
                nc.sync.dma_start(wv[:, ct, cs], wv_d[ct][:, cs])
        for ct in range(CT):
            nc.sync.dma_start(xt[:, ct, 512:N], xT_d[ct * P:(ct + 1) * P, 512:N])
        for t in range(1, TP):
            for s, wsrc in ((0, wq_d), (1, wk_d)):
                nc.sync.dma_start(wqk[:, t, s], wsrc[t].rearrange("a p c -> p a c"))
        wp = const.tile([P, CT, C], bf16)
        for ct in range(CT):
            nc.sync.dma_start(wp[:, ct, :], wp_d[ct])
        bias_bc = const.tile([P, C], f32)
        nc.sync.dma_start(bias_bc[:1, :], bias_d[:])
        nc.gpsimd.partition_broadcast(bias_bc[:], bias_bc[:1, :])
        sel2 = const.tile([2, P], bf16)
        nc.sync.dma_start(sel2[:], sel2_d[:])

        # v storage: [128 j-local, NT j-tiles, H heads x (64 v + 1 ones col)]
        v_all = vpool.tile([P, NT, H * (HD + 1)], bf16)
        ones_c = const.tile([P, H], f32)
        nc.vector.memset(ones_c[:], 1.0)
        for jt in range(NT):
            nc.vector.tensor_copy(
                v_all[:, jt, :].rearrange("p (h c) -> p h c", c=HD + 1)[:, :, HD:],
                ones_c[:].rearrange("p (h o) -> p h o", o=1))
        # out.T accumulator: [128 c-local, CT c-tiles, 1024 n]
        outT = otpool.tile([P, CT, N], bf16)

        qk_fin = {}
        spairs = {}
        v_ready = set()

        def prepare_pair(t):
            """Generator: qk D-tile matmuls + RoPE for pair t, yielding after
            each PE instruction so it can interleave into attention."""
            fins = []
            for s in range(2):  # 0 = q, 1 = k
                pre = qkpre.tile([P, N], bf16, tag="pre", name="pre")
                for ch in range(2):
                    qps = mi_ps.tile([P, 512], f32, tag="mi", name="qps")
                    for ct in range(CT):
                        nc.tensor.matmul(
                            qps[:],
                            wqk[:, t, s, ct],
                            xt[:, ct, ch * 512:(ch + 1) * 512],
                            start=(ct == 0), stop=(ct == CT - 1),
                        )
                        yield
                    nc.vector.tensor_copy(pre[:, ch * 512:(ch + 1) * 512], qps[:])
                rot = qkpre.tile([P, N], bf16, tag="rot", name="rot")
                nc.vector.stream_shuffle(rot[:], pre[:], SHUF_MASK)
                fin = qkfin.tile([P, N], bf16, tag="fin", name="fin")
                tmp = qkpre.tile([P, N], bf16, tag="tmp", name="tmp")
                nc.vector.tensor_mul(tmp[:], pre[:], cos2[:])
                nc.vector.tensor_mul(fin[:], rot[:], sin2[:])
                nc.vector.tensor_add(fin[:], fin[:], tmp[:])
                fins.append(fin)
            qk_fin[t] = fins

        def v_group(nt, ch):
            """Generator: one v-projection group (8 accumulating matmuls +
            copy into the packed v_all layout)."""
            vps = mi_ps.tile([P, 512], f32, tag="mi", name="vps")
            for ct in range(CT):
                nc.tensor.matmul(
                    vps[:],
                    xt[:, ct, nt * P:(nt + 1) * P],
                    wv[:, ct, ch * 512:(ch + 1) * 512],
                    start=(ct == 0), stop=(ct == CT - 1),
                )
                yield
            nc.vector.tensor_copy(
                v_all[:, nt, :].rearrange(
                    "p (h c) -> p h c", c=HD + 1)[:, 8 * ch:8 * ch + 8, :HD],
                vps[:])
            v_ready.add((nt, ch))

        def chain(*gens):
            for g in gens:
                yield from g

        def pull(feeder, k):
            if feeder is None:
                return None
            for _ in range(k):
                if next(feeder, "done") == "done":
                    return None
            return feeder

        def ensure_v(feeder, nt, vch):
            """Drain the feeder until v_group(nt, vch) has been emitted."""
            while (nt, vch) not in v_ready:
                assert feeder is not None, f"v_group({nt},{vch}) unreachable"
                feeder = pull(feeder, 1)
            return feeder

        def attention(t, feeder, npull):
            """Attention for head pair t (heads 2t, 2t+1), feeder interleaved."""
            qf, kf = qk_fin.pop(t)
            hA, hB = 2 * t, 2 * t + 1
            spair = nrmp.tile([2, N], f32, tag="spair", name="spair")
            spairs[t] = spair
            for ch in range(2):
                cs = slice(ch * 512, (ch + 1) * 512)
                avA = av_ps.tile([HD + 1, 512], f32, tag="av", name="avA")
                avB = av_ps.tile([HD + 1, 512], f32, tag="av", name="avB")
                pend = None  # (ex, jt) awaiting av matmuls
                for jt in range(NT):
                    sT = sT_ps.tile([P, N], f32, tag="sT", name="sT")
                    js = slice(jt * P, (jt + 1) * P)
                    # two K=64 row-tiles, back-to-back -> concurrent on PE
                    nc.tensor.matmul(sT[:, 0:512], kf[0:64, js], qf[0:64, cs],
                                     start=True, stop=True)
                    nc.tensor.matmul(sT[:, 512:1024], kf[64:128, js],
                                     qf[64:128, cs], start=True, stop=True)
                    ex = expp.tile([P, N], bf16, tag="ex", name="ex")
                    nc.scalar.activation(ex[:], sT[:], EXP, scale=SCALE)
                    feeder = pull(feeder, npull)
                    if pend is not None:
                        pex, pjt = pend
                        feeder = ensure_v(feeder, pjt, t // 4)
                        nc.tensor.matmul(
                            avA[:], v_all[:, pjt, hA * (HD + 1):(hA + 1) * (HD + 1)],
                            pex[:, 0:512], start=(pjt == 0), stop=False)
                        nc.tensor.matmul(
                            avB[:], v_all[:, pjt, hB * (HD + 1):(hB + 1) * (HD + 1)],
                            pex[:, 512:1024], start=(pjt == 0), stop=False)
                    pend = (ex, jt)
                pex, pjt = pend
                feeder = ensure_v(feeder, pjt, t // 4)
                nc.tensor.matmul(
                    avA[:], v_all[:, pjt, hA * (HD + 1):(hA + 1) * (HD + 1)],
                    pex[:, 0:512], start=False, stop=True)
                nc.tensor.matmul(
                    avB[:], v_all[:, pjt, hB * (HD + 1):(hB + 1) * (HD + 1)],
                    pex[:, 512:1024], start=False, stop=True)
                # rows 0:64 = unnormalized out.T; row 64 = softmax sums
                nc.vector.tensor_copy(outT[0:64, t, cs], avA[:HD, :])
                nc.vector.tensor_copy(outT[64:128, t, cs], avB[:HD, :])
                # sums rows leave PSUM via ACT (DVE is busier), then SBUF DMA
                # assembles them on partitions 0/1 of spair
                cpA = cpsp.tile([1, 512], f32, tag="cp", name="cpA")
                nc.scalar.activation(cpA[:], avA[HD:HD + 1, :], CPY)
                nc.sync.dma_start(spair[0:1, cs], cpA[:])
                cpB = cpsp.tile([1, 512], f32, tag="cp", name="cpB")
                nc.scalar.activation(cpB[:], avB[HD:HD + 1, :], CPY)
                nc.sync.dma_start(spair[1:2, cs], cpB[:])
            return feeder

        def normalize(t):
            """Per-pair softmax normalization of outT[:, t, :]: K=2 selection
            matmul broadcasts each head's reciprocal sum over its 64 rows."""
            spair = spairs.pop(t)
            rcp2 = nrmp.tile([2, N], f32, tag="rcp2", name="rcp2")
            nc.vector.reciprocal_approx_fast(rcp2[:], spair[:])
            rcp16 = nrmp.tile([2, N], bf16, tag="rcp16", name="rcp16")
            nc.vector.tensor_copy(rcp16[:], rcp2[:])
            for ch in range(2):
                cs = slice(ch * 512, (ch + 1) * 512)
                rb = mi_ps.tile([P, 512], f32, tag="mi", name="rb")
                nc.tensor.matmul(rb[:], sel2[:], rcp16[:, cs],
                                 start=True, stop=True)
                nc.vector.tensor_mul(outT[:, t, cs], outT[:, t, cs], rb[:])

        def proj_group(nt, ch2, ct_hi):
            """Emit out-proj accumulation matmuls for cts [0, ct_hi)."""
            yps = mi_ps.tile([P, 512], f32, tag="mi", name="yps")
            es = slice(ch2 * 512, (ch2 + 1) * 512)
            ns = slice(nt * P, (nt + 1) * P)
            for ct in range(ct_hi):
                nc.tensor.matmul(
                    yps[:], outT[:, ct, ns], wp[:, ct, es],
                    start=(ct == 0), stop=False,
                )
            return yps

        def proj_close(nt, ch2, yps):
            es = slice(ch2 * 512, (ch2 + 1) * 512)
            ns = slice(nt * P, (nt + 1) * P)
            nc.tensor.matmul(
                yps[:], outT[:, CT - 1, ns], wp[:, CT - 1, es],
                start=False, stop=True,
            )
            yb = ybp.tile([P, 512], f32, tag="yb", name="yb")
            nc.vector.tensor_add(yb[:], yps[:], bias_bc[:, es])
            nc.sync.dma_start(out_d[ns, es], yb[:])

        # ---- prologue: pair-0 qk + first v groups ----
        pull(chain(prepare_pair(0), v_group(0, 0), v_group(1, 0)), 10 ** 6)

        # feeder work for each pair's attention window
        feeders = [
            chain(v_group(2, 0), v_group(3, 0), v_group(4, 0), v_group(5, 0),
                  v_group(6, 0), v_group(7, 0), prepare_pair(1)),
            chain(v_group(0, 1), v_group(1, 1), v_group(2, 1), prepare_pair(2)),
            chain(v_group(3, 1), v_group(4, 1), v_group(5, 1), prepare_pair(3)),
            chain(v_group(6, 1), v_group(7, 1), prepare_pair(4)),
            prepare_pair(5),
            prepare_pair(6),
            prepare_pair(7),
            None,
        ]
        npulls = [6, 4, 4, 4, 3, 3, 3, 0]

        for t in range(TP):
            feeder = attention(t, feeders[t], npulls[t])
            pull(feeder, 10 ** 6)
            if t >= 1:
                # normalize the PREVIOUS pair: its sums DMAs have had a full
                # pair window to land, so the chain is off the critical path
                normalize(t - 1)
        normalize(TP - 1)

        # ---- out-proj: y = outT.T @ w_projT + bias ----
        open_groups = [(0, 0, proj_group(0, 0, CT - 1)),
                       (0, 1, proj_group(0, 1, CT - 1))]
        for nt, ch2, yps in open_groups:
            proj_close(nt, ch2, yps)
        for nt in range(NT):
            for ch2 in range(2):
                if nt == 0:
                    continue
                proj_close(nt, ch2, proj_group(nt, ch2, CT - 1))

    nc.compile()
    return nc


def get_nc():
    if "nc" not in _CACHE:
        _CACHE["nc"] = _build_nc()
    return _CACHE["nc"]


def _host_inputs(x, xpos, w_qkv, w_proj, b_proj):
    """Host-side reshapes: transposes, RoPE tables, weight packing."""
    import ml_dtypes

    x = np.asarray(x, dtype=np.float32)
    xpos = np.asarray(xpos)
    w_qkv = np.asarray(w_qkv, dtype=np.float32)
    w_proj = np.asarray(w_proj, dtype=np.float32)
    b_proj = np.asarray(b_proj, dtype=np.float32).reshape(1, C)

    xT = np.ascontiguousarray(x.transpose(0, 2, 1))  # [B, C, N]

    # RoPE tables in [d, n] orientation, two head-copies stacked to 128 rows.
    inv_freq = (100.0 ** (-np.arange(16, dtype=np.float64) / 16.0))
    py = xpos[..., 0].astype(np.float64)  # [B, N]
    px = xpos[..., 1].astype(np.float64)
    angy = py[:, :, None] * inv_freq      # [B, N, 16]
    angx = px[:, :, None] * inv_freq
    cos64 = np.concatenate(
        [np.cos(angy), np.cos(angy), np.cos(angx), np.cos(angx)], axis=2)
    sin64 = np.concatenate(
        [-np.sin(angy), np.sin(angy), -np.sin(angx), np.sin(angx)], axis=2)
    cos2 = np.ascontiguousarray(
        np.tile(cos64, (1, 1, 2)).transpose(0, 2, 1)).astype(np.float32)
    sin2 = np.ascontiguousarray(
        np.tile(sin64, (1, 1, 2)).transpose(0, 2, 1)).astype(np.float32)

    wq = np.zeros((TP, CT, P, P), dtype=np.float32)
    wk = np.zeros((TP, CT, P, P), dtype=np.float32)
    for t in range(TP):
        for ct in range(CT):
            wq[t, ct] = w_qkv[t * P:(t + 1) * P, ct * P:(ct + 1) * P].T
            wk[t, ct] = w_qkv[C + t * P:C + (t + 1) * P, ct * P:(ct + 1) * P].T
    wv = np.ascontiguousarray(
        w_qkv[2 * C:3 * C, :].T.reshape(CT, P, C))   # [ct][c-local, dd]
    wp = np.ascontiguousarray(w_proj.T.reshape(CT, P, C))  # [ct][c-local, e]

    sel2 = np.zeros((2, P), dtype=np.float32)
    sel2[0, :HD] = 1.0
    sel2[1, HD:] = 1.0

    def mcast(a):
        return np.ascontiguousarray(a).astype(ml_dtypes.bfloat16)

    shared = dict(wq=mcast(wq), wk=mcast(wk), wv=mcast(wv), wp=mcast(wp),
                  sel2=mcast(sel2), bias=b_proj)
    in_maps = []
    for b in range(B):
        m = dict(shared)
        m["xT"] = mcast(xT[b])
        m["cos2"] = mcast(cos2[b])
        m["sin2"] = mcast(sin2[b])
        in_maps.append(m)
    return in_maps


def kernel(x, xpos, w_qkv, w_proj, b_proj):
    from concourse import bass_utils

    nc = get_nc()
    in_maps = _host_inputs(x, xpos, w_qkv, w_proj, b_proj)
    res = bass_utils.run_bass_kernel_spmd(
        nc, in_maps, core_ids=list(range(B)),
        trace=bool(int(os.environ.get("BASS_ATTN_TRACE", "0"))),
    )
    out = np.stack([res.results[b]["out"] for b in range(B)], axis=0)
    _CACHE["last_results"] = res
    return out


# revision 23
# speedup vs baseline: 1.1945x; 1.0720x over previous
"""Fused multi-head attention (QKV + RoPE2D + softmax + out-proj) on 8 TRN2 cores.

Sharding: batch-parallel. B == n_cores == 8, so each core runs one batch
element end-to-end; weights are replicated. No collectives needed.

Per-core dataflow (matmul operands in bf16, accumulation in f32 PSUM):
  qkT[D,n] = (x @ w_{q,k}.T).T   (lhsT = w chunks, rhs = xT tiles)
  RoPE on qkT: rot = stream_shuffle(qkT) (lane permute i^16 per 32-block),
  qk' = qkT*cos2 + rot*sin2      (signs folded into sin2 host-side; all bf16)
  v[n,dd] = x @ w_v.T            (v-proj groups streamed inside the feeder)
  attention per head-PAIR t (heads A=2t on partitions 0:64 of qk', B=2t+1
  on 64:128): per (ch, jt):
     sT[:, 0:512]   = kA-tile.T @ qA-chunk   (K=64, PE rows 0-63)
     sT[:, 512:1024]= kB-tile.T @ qB-chunk   (K=64, PE rows 64-127)
     -> issued back-to-back, the two row-tiles stream CONCURRENTLY
     ex = exp(sT/8) on ACT, one [128,1024] call for both heads
     av_X += [v_X | ones].T @ ex[:, X-half]  (K=128, accumulated over jt,
              software-pipelined one step behind exp so PE never waits)
  softmax sums ride in av row 64; per-pair normalization: ACT copies the
  sums rows out of PSUM, DVE reciprocal, GPSIMD partition-broadcasts the
  per-head scales to 64 rows, one DVE multiply normalizes outT[:, t, :].
  y = outT.T @ w_projT (+bias via DVE add) -> DMA out; ct accumulation in
  pair-completion order so only pair 7's normalization is on the tail.

The next pair's QKV matmuls and the v-projection groups are interleaved
into the current pair's attention (generator-based software pipelining)
so the in-order PE queue always has ready work during exp waits.
"""

import os
import numpy as np

B, N, C = 8, 1024, 1024
H, HD = 16, 64
P = 128
NT = N // P          # 8 n-tiles
CT = C // P          # 8 c-tiles
TP = H // 2          # 8 head-pairs (qk D-tiles per q/k)
SCALE = HD ** -0.5   # 1/8

_CACHE = {}

SHUF_MASK = [i ^ 16 for i in range(32)]  # rotate_half as 32-lane permute


def _build_nc():
    import concourse.mybir as mybir
    from concourse import bacc, tile
    from contextlib import ExitStack

    f32 = mybir.dt.float32
    bf16 = mybir.dt.bfloat16
    EXP = mybir.ActivationFunctionType.Exp
    CPY = mybir.ActivationFunctionType.Copy

    nc = bacc.Bacc(
        "TRN2", target_bir_lowering=False, debug=False,
        enable_asserts=False, num_devices=B,
    )

    xT_d = nc.dram_tensor("xT", [C, N], bf16, kind="ExternalInput")
    cos2_d = nc.dram_tensor("cos2", [P, N], bf16, kind="ExternalInput")
    sin2_d = nc.dram_tensor("sin2", [P, N], bf16, kind="ExternalInput")
    # wqk pre-transposed host-side into the SBUF layout: [t, s, p, ct*P]
    wqk_d = nc.dram_tensor("wqk", [TP, 2, P, CT * P], bf16, kind="ExternalInput")
    wv_d = nc.dram_tensor("wv", [CT, P, C], bf16, kind="ExternalInput")
    wp_d = nc.dram_tensor("wp", [CT, P, C], bf16, kind="ExternalInput")
    sel2_d = nc.dram_tensor("sel2", [2, P], bf16, kind="ExternalInput")
    bias_d = nc.dram_tensor("bias", [1, C], f32, kind="ExternalInput")
    out_d = nc.dram_tensor("out", [N, C], f32, kind="ExternalOutput")

    with tile.TileContext(nc) as tc, ExitStack() as ctx:
        const = ctx.enter_context(tc.tile_pool(name="const", bufs=1))
        vpool = ctx.enter_context(tc.tile_pool(name="vpool", bufs=1))
        otpool = ctx.enter_context(tc.tile_pool(name="otpool", bufs=1))
        qkpre = ctx.enter_context(tc.tile_pool(name="qkpre", bufs=2))
        qkfin = ctx.enter_context(tc.tile_pool(name="qkfin", bufs=4))
        expp = ctx.enter_context(tc.tile_pool(name="expp", bufs=3))
        cpsp = ctx.enter_context(tc.tile_pool(name="cpsp", bufs=4))
        nrmp = ctx.enter_context(tc.tile_pool(name="nrmp", bufs=2))
        ybp = ctx.enter_context(tc.tile_pool(name="ybp", bufs=3))
        sT_ps = ctx.enter_context(tc.tile_pool(name="sT_ps", bufs=2, space="PSUM"))
        av_ps = ctx.enter_context(tc.tile_pool(name="av_ps", bufs=2, space="PSUM"))
        mi_ps = ctx.enter_context(tc.tile_pool(name="mi_ps", bufs=2, space="PSUM"))

        # ---- input DMA, strict priority order (sync engine issues in order,
        # so the prologue's dependencies come first, bulk weights later) ----
        wqk = const.tile([P, TP, 2, CT, P], bf16)  # all q,k weight D-tiles
        for s in range(2):
            nc.sync.dma_start(
                wqk[:, 0, s].rearrange("p ct c -> p (ct c)"), wqk_d[0, s])
        xt = const.tile([P, CT, N], bf16)          # xT tiles, ct-indexed
        nc.sync.dma_start(
            xt[:, :, 0:512],
            xT_d[:, 0:512].rearrange("(ct p) n -> p ct n", p=P))
        cos2 = const.tile([P, N], bf16)
        nc.sync.dma_start(cos2[:], cos2_d[:])
        sin2 = const.tile([P, N], bf16)
        nc.sync.dma_start(sin2[:], sin2_d[:])
        nc.sync.dma_start(
            xt[:, :, 512:N],
            xT_d[:, 512:N].rearrange("(ct p) n -> p ct n", p=P))
        wv = const.tile([P, CT, C], bf16)
        nc.sync.dma_start(
            wv[:, :, 0:512],
            wv_d[:, :, 0:512].rearrange("ct p c -> p ct c"))
        for t in range(1, TP):
            nc.sync.dma_start(
                wqk[:, t].rearrange("p s ct c -> p s (ct c)"),
                wqk_d[t].rearrange("s p c -> p s c"))
        nc.sync.dma_start(
            wv[:, :, 512:C],
            wv_d[:, :, 512:C].rearrange("ct p c -> p ct c"))
        wp = const.tile([P, CT, C], bf16)
        nc.sync.dma_start(wp[:], wp_d[:].rearrange("ct p c -> p ct c"))
        bias_bc = const.tile([P, C], f32)
        nc.sync.dma_start(bias_bc[:1, :], bias_d[:])
        nc.gpsimd.partition_broadcast(bias_bc[:], bias_bc[:1, :])
        sel2 = const.tile([2, P], bf16)
        nc.sync.dma_start(sel2[:], sel2_d[:])

        # v storage: [128 j-local, NT j-tiles, H heads x (64 v + 1 ones col)]
        v_all = vpool.tile([P, NT, H * (HD + 1)], bf16)
        ones_c = const.tile([P, H], f32)
        nc.vector.memset(ones_c[:], 1.0)
        for jt in range(NT):
            nc.vector.tensor_copy(
                v_all[:, jt, :].rearrange("p (h c) -> p h c", c=HD + 1)[:, :, HD:],
                ones_c[:].rearrange("p (h o) -> p h o", o=1))
        # out.T accumulator: [128 c-local, CT c-tiles, 1024 n]
        outT = otpool.tile([P, CT, N], bf16)
        # normalization staging: reciprocal rows land on partitions
        # 0/32/64/96; the rest must be initialized for the shuffle's read
        st_bc = otpool.tile([P, N], f32)
        nc.vector.memset(st_bc[:], 1.0)

        qk_fin = {}
        spairs = {}
        v_ready = set()

        def prepare_pair(t):
            """Generator: qk D-tile matmuls + RoPE for pair t, yielding after
            each PE instruction so it can interleave into attention."""
            fins = []
            for s in range(2):  # 0 = q, 1 = k
                pre = qkpre.tile([P, N], bf16, tag="pre", name="pre")
                for ch in range(2):
                    qps = mi_ps.tile([P, 512], f32, tag="mi", name="qps")
                    for ct in range(CT):
                        nc.tensor.matmul(
                            qps[:],
                            wqk[:, t, s, ct],
                            xt[:, ct, ch * 512:(ch + 1) * 512],
                            start=(ct == 0), stop=(ct == CT - 1),
                        )
                        yield
                    nc.vector.tensor_copy(pre[:, ch * 512:(ch + 1) * 512], qps[:])
                rot = qkpre.tile([P, N], bf16, tag="rot", name="rot")
                nc.vector.stream_shuffle(rot[:], pre[:], SHUF_MASK)
                fin = qkfin.tile([P, N], bf16, tag="fin", name="fin")
                tmp = qkpre.tile([P, N], bf16, tag="tmp", name="tmp")
                nc.vector.tensor_mul(tmp[:], pre[:], cos2[:])
                nc.vector.tensor_mul(fin[:], rot[:], sin2[:])
                nc.vector.tensor_add(fin[:], fin[:], tmp[:])
                fins.append(fin)
            qk_fin[t] = fins

        def v_group(nt, ch):
            """Generator: one v-projection group (8 accumulating matmuls +
            copy into the packed v_all layout)."""
            vps = mi_ps.tile([P, 512], f32, tag="mi", name="vps")
            for ct in range(CT):
                nc.tensor.matmul(
                    vps[:],
                    xt[:, ct, nt * P:(nt + 1) * P],
                    wv[:, ct, ch * 512:(ch + 1) * 512],
                    start=(ct == 0), stop=(ct == CT - 1),
                )
                yield
            nc.vector.tensor_copy(
                v_all[:, nt, :].rearrange(
                    "p (h c) -> p h c", c=HD + 1)[:, 8 * ch:8 * ch + 8, :HD],
                vps[:])
            v_ready.add((nt, ch))

        def chain(*gens):
            for g in gens:
                yield from g

        def pull(feeder, k):
            if feeder is None:
                return None
            for _ in range(k):
                if next(feeder, "done") == "done":
                    return None
            return feeder

        def ensure_v(feeder, nt, vch):
            """Drain the feeder until v_group(nt, vch) has been emitted."""
            while (nt, vch) not in v_ready:
                assert feeder is not None, f"v_group({nt},{vch}) unreachable"
                feeder = pull(feeder, 1)
            return feeder

        def attention(t, feeder, npull):
            """Attention for head pair t (heads 2t, 2t+1), feeder interleaved."""
            qf, kf = qk_fin.pop(t)
            hA, hB = 2 * t, 2 * t + 1
            spair = nrmp.tile([2, N], f32, tag="spair", name="spair")
            spairs[t] = spair
            for ch in range(2):
                cs = slice(ch * 512, (ch + 1) * 512)
                avA = av_ps.tile([HD + 1, 512], f32, tag="av", name="avA")
                avB = av_ps.tile([HD + 1, 512], f32, tag="av", name="avB")
                pend = None  # (ex, jt) awaiting av matmuls
                for jt in range(NT):
                    sT = sT_ps.tile([P, N], f32, tag="sT", name="sT")
                    js = slice(jt * P, (jt + 1) * P)
                    # two K=64 row-tiles, back-to-back -> concurrent on PE
                    nc.tensor.matmul(sT[:, 0:512], kf[0:64, js], qf[0:64, cs],
                                     start=True, stop=True)
                    nc.tensor.matmul(sT[:, 512:1024], kf[64:128, js],
                                     qf[64:128, cs], start=True, stop=True)
                    ex = expp.tile([P, N], bf16, tag="ex", name="ex")
                    nc.scalar.activation(ex[:], sT[:], EXP, scale=SCALE)
                    feeder = pull(feeder, npull)
                    if pend is not None:
                        pex, pjt = pend
                        feeder = ensure_v(feeder, pjt, t // 4)
                        nc.tensor.matmul(
                            avA[:], v_all[:, pjt, hA * (HD + 1):(hA + 1) * (HD + 1)],
                            pex[:, 0:512], start=(pjt == 0), stop=False)
                        nc.tensor.matmul(
                            avB[:], v_all[:, pjt, hB * (HD + 1):(hB + 1) * (HD + 1)],
                            pex[:, 512:1024], start=(pjt == 0), stop=False)
                    pend = (ex, jt)
                pex, pjt = pend
                feeder = ensure_v(feeder, pjt, t // 4)
                nc.tensor.matmul(
                    avA[:], v_all[:, pjt, hA * (HD + 1):(hA + 1) * (HD + 1)],
                    pex[:, 0:512], start=False, stop=True)
                nc.tensor.matmul(
                    avB[:], v_all[:, pjt, hB * (HD + 1):(hB + 1) * (HD + 1)],
                    pex[:, 512:1024], start=False, stop=True)
                # rows 0:64 = unnormalized out.T; row 64 = softmax sums
                nc.vector.tensor_copy(outT[0:64, t, cs], avA[:HD, :])
                nc.vector.tensor_copy(outT[64:128, t, cs], avB[:HD, :])
                # sums rows leave PSUM on DVE (ACT must stay exp-only: it is
                # the attention pacer), then SBUF DMA assembles them on
                # partitions 0/1 of spair
                cpA = cpsp.tile([1, 512], f32, tag="cp", name="cpA")
                nc.vector.tensor_copy(cpA[:], avA[HD:HD + 1, :])
                nc.sync.dma_start(spair[0:1, cs], cpA[:])
                cpB = cpsp.tile([1, 512], f32, tag="cp", name="cpB")
                nc.vector.tensor_copy(cpB[:], avB[HD:HD + 1, :])
                nc.sync.dma_start(spair[1:2, cs], cpB[:])
            return feeder

        BCAST_MASK = [0] * 32

        def normalize_gp(t):
            """Per-pair softmax normalization of outT[:, t, :] — used for
            pairs whose normalization has a full pair window of slack.
            DMA stages the reciprocal rows at partitions 0/32/64/96, one
            stream_shuffle broadcasts within each 32-block, and the multiply
            runs on the otherwise-idle GPSIMD."""
            spair = spairs.pop(t)
            rcp2 = nrmp.tile([2, N], f32, tag="rcp2", name="rcp2")
            nc.vector.reciprocal_approx_fast(rcp2[:], spair[:])
            for q, r in ((0, 0), (32, 0), (64, 1), (96, 1)):
                nc.sync.dma_start(st_bc[q:q + 1, :], rcp2[r:r + 1, :])
            rb = nrmp.tile([P, N], f32, tag="rb", name="rb")
            nc.vector.stream_shuffle(rb[:], st_bc[:], BCAST_MASK)
            nc.gpsimd.tensor_mul(outT[:, t, :], outT[:, t, :], rb[:])

        def normalize_sel(t):
            """Pair-7 normalization: K=2 selection matmul + DVE multiplies —
            shortest latency chain for the tail (rb rides the freed sT tag)."""
            spair = spairs.pop(t)
            rcp2 = nrmp.tile([2, N], f32, tag="rcp2", name="rcp2")
            nc.vector.reciprocal_approx_fast(rcp2[:], spair[:])
            rcp16 = nrmp.tile([2, N], bf16, tag="rcp16", name="rcp16")
            nc.vector.tensor_copy(rcp16[:], rcp2[:])
            for ch in range(2):
                cs = slice(ch * 512, (ch + 1) * 512)
                rb = sT_ps.tile([P, 512], f32, tag="sT", name="rbps")
                nc.tensor.matmul(rb[:], sel2[:], rcp16[:, cs],
                                 start=True, stop=True)
                nc.vector.tensor_mul(outT[:, t, cs], outT[:, t, cs], rb[:])

        def proj_group(nt, ch2, ct_hi):
            """Emit out-proj accumulation matmuls for cts [0, ct_hi)."""
            yps = mi_ps.tile([P, 512], f32, tag="mi", name="yps")
            es = slice(ch2 * 512, (ch2 + 1) * 512)
            ns = slice(nt * P, (nt + 1) * P)
            for ct in range(ct_hi):
                nc.tensor.matmul(
                    yps[:], outT[:, ct, ns], wp[:, ct, es],
                    start=(ct == 0), stop=False,
                )
            return yps

        def proj_close(nt, ch2, yps):
            es = slice(ch2 * 512, (ch2 + 1) * 512)
            ns = slice(nt * P, (nt + 1) * P)
            nc.tensor.matmul(
                yps[:], outT[:, CT - 1, ns], wp[:, CT - 1, es],
                start=False, stop=True,
            )
            yb = ybp.tile([P, 512], f32, tag="yb", name="yb")
            nc.vector.tensor_add(yb[:], yps[:], bias_bc[:, es])
            nc.sync.dma_start(out_d[ns, es], yb[:])

        # ---- prologue: pair-0 qk + first v groups ----
        pull(chain(prepare_pair(0), v_group(0, 0), v_group(1, 0)), 10 ** 6)

        # feeder work for each pair's attention window
        feeders = [
            chain(v_group(2, 0), v_group(3, 0), v_group(4, 0), v_group(5, 0),
                  v_group(6, 0), v_group(7, 0), prepare_pair(1)),
            chain(v_group(0, 1), v_group(1, 1), v_group(2, 1), prepare_pair(2)),
            chain(v_group(3, 1), v_group(4, 1), v_group(5, 1), prepare_pair(3)),
            chain(v_group(6, 1), v_group(7, 1), prepare_pair(4)),
            prepare_pair(5),
            prepare_pair(6),
            prepare_pair(7),
            None,
        ]
        npulls = [6, 4, 4, 4, 3, 3, 3, 0]

        for t in range(TP):
            feeder = attention(t, feeders[t], npulls[t])
            pull(feeder, 10 ** 6)
            if t >= 1:
                # normalize the PREVIOUS pair: its sums DMAs have had a full
                # pair window to land, so the chain is off the critical path
                normalize_gp(t - 1)

        # ---- out-proj: y = outT.T @ w_projT + bias ----
        # prefill two groups' ct 0..6 before pair-7 normalization is emitted
        # so the PE has work during the normalization chain
        open_groups = [(0, 0, proj_group(0, 0, CT - 1)),
                       (0, 1, proj_group(0, 1, CT - 1))]
        normalize_sel(TP - 1)
        for nt, ch2, yps in open_groups:
            proj_close(nt, ch2, yps)
        for nt in range(NT):
            for ch2 in range(2):
                if nt == 0:
                    continue
                proj_close(nt, ch2, proj_group(nt, ch2, CT - 1))

    nc.compile()
    return nc


def get_nc():
    if "nc" not in _CACHE:
        _CACHE["nc"] = _build_nc()
    return _CACHE["nc"]


def _host_inputs(x, xpos, w_qkv, w_proj, b_proj):
    """Host-side reshapes: transposes, RoPE tables, weight packing."""
    import ml_dtypes

    x = np.asarray(x, dtype=np.float32)
    xpos = np.asarray(xpos)
    w_qkv = np.asarray(w_qkv, dtype=np.float32)
    w_proj = np.asarray(w_proj, dtype=np.float32)
    b_proj = np.asarray(b_proj, dtype=np.float32).reshape(1, C)

    xT = np.ascontiguousarray(x.transpose(0, 2, 1))  # [B, C, N]

    # RoPE tables in [d, n] orientation, two head-copies stacked to 128 rows.
    inv_freq = (100.0 ** (-np.arange(16, dtype=np.float64) / 16.0))
    py = xpos[..., 0].astype(np.float64)  # [B, N]
    px = xpos[..., 1].astype(np.float64)
    angy = py[:, :, None] * inv_freq      # [B, N, 16]
    angx = px[:, :, None] * inv_freq
    cos64 = np.concatenate(
        [np.cos(angy), np.cos(angy), np.cos(angx), np.cos(angx)], axis=2)
    sin64 = np.concatenate(
        [-np.sin(angy), np.sin(angy), -np.sin(angx), np.sin(angx)], axis=2)
    cos2 = np.ascontiguousarray(
        np.tile(cos64, (1, 1, 2)).transpose(0, 2, 1)).astype(np.float32)
    sin2 = np.ascontiguousarray(
        np.tile(sin64, (1, 1, 2)).transpose(0, 2, 1)).astype(np.float32)

    # wqk[t, s] = [c-local partition, ct*P + d] — the exact SBUF layout, so
    # the weight DMA is a single contiguous 2KB-per-partition transfer
    wqk = np.zeros((TP, 2, P, CT * P), dtype=np.float32)
    for t in range(TP):
        for s in range(2):
            rows = w_qkv[s * C + t * P:s * C + (t + 1) * P, :]  # [P(d), C]
            wqk[t, s] = rows.reshape(P, CT, P).transpose(2, 1, 0).reshape(
                P, CT * P)
    wv = np.ascontiguousarray(
        w_qkv[2 * C:3 * C, :].T.reshape(CT, P, C))   # [ct][c-local, dd]
    wp = np.ascontiguousarray(w_proj.T.reshape(CT, P, C))  # [ct][c-local, e]

    sel2 = np.zeros((2, P), dtype=np.float32)
    sel2[0, :HD] = 1.0
    sel2[1, HD:] = 1.0

    def mcast(a):
        return np.ascontiguousarray(a).astype(ml_dtypes.bfloat16)

    shared = dict(wqk=mcast(wqk), wv=mcast(wv), wp=mcast(wp),
                  sel2=mcast(sel2), bias=b_proj)
    in_maps = []
    for b in range(B):
        m = dict(shared)
        m["xT"] = mcast(xT[b])
        m["cos2"] = mcast(cos2[b])
        m["sin2"] = mcast(sin2[b])
        in_maps.append(m)
    return in_maps


def kernel(x, xpos, w_qkv, w_proj, b_proj):
    from concourse import bass_utils

    nc = get_nc()
    in_maps = _host_inputs(x, xpos, w_qkv, w_proj, b_proj)
    res = bass_utils.run_bass_kernel_spmd(
        nc, in_maps, core_ids=list(range(B)),
        trace=bool(int(os.environ.get("BASS_ATTN_TRACE", "0"))),
    )
    out = np.stack([res.results[b]["out"] for b in range(B)], axis=0)
    _CACHE["last_results"] = res
    return out


# revision 30
# speedup vs baseline: 1.2231x; 1.0240x over previous
"""Fused multi-head attention (QKV + RoPE2D + softmax + out-proj) on 8 TRN2 cores.

Sharding: batch-parallel. B == n_cores == 8, so each core runs one batch
element end-to-end; weights are replicated. No collectives needed.

Per-core dataflow (matmul operands in bf16, accumulation in f32 PSUM):
  qkT[D,n] = (x @ w_{q,k}.T).T   (lhsT = w chunks, rhs = xT tiles)
  RoPE on qkT: rot = stream_shuffle(qkT) (lane permute i^16 per 32-block),
  qk' = qkT*cos2 + rot*sin2      (signs folded into sin2 host-side; all bf16)
  v[n,dd] = x @ w_v.T            (v-proj groups streamed inside the feeder)
  attention per head-PAIR t (heads A=2t on partitions 0:64 of qk', B=2t+1
  on 64:128), one flat software pipeline over (t, ch, jt):
     sT[:, 0:512]   = kA-tile.T @ qA-chunk   (K=64, PE rows 0-63)
     sT[:, 512:1024]= kB-tile.T @ qB-chunk   (K=64, PE rows 64-127)
     -> issued back-to-back, the two row-tiles stream CONCURRENTLY
     ex = exp(sT/8) on ACT, one [128,1024] call for both heads
     av_X += [v_X | ones].T @ ex[:, X-half]  (K=128, accumulated over jt)
  The av matmuls trail the exp by one pipeline step ACROSS ch and pair
  boundaries, so the in-order PE queue never blocks on the ACT engine.
  Softmax sums ride in av row 64; per-pair normalization (reciprocal ->
  stream_shuffle broadcast -> GPSIMD multiply) is emitted one pair late so
  its chain is off the critical path; pair 7 uses a K=2 selection-matmul
  chain (lowest latency) plus a prefilled out-projection to bury the tail.
  y = outT.T @ w_projT (+bias via DVE add) -> DMA out.

The next pair's QKV matmuls and the v-projection groups are interleaved
into the current pair's attention (generator-based software pipelining)
so the in-order PE queue always has ready work during exp waits.
All bulk inputs are host-packed into their exact SBUF layouts so every
input DMA is a contiguous >=2KB-per-partition transfer.
"""

import os
import numpy as np

B, N, C = 8, 1024, 1024
H, HD = 16, 64
P = 128
NT = N // P          # 8 n-tiles
CT = C // P          # 8 c-tiles
TP = H // 2          # 8 head-pairs (qk D-tiles per q/k)
SCALE = HD ** -0.5   # 1/8

_CACHE = {}

SHUF_MASK = [i ^ 16 for i in range(32)]   # rotate_half as 32-lane permute
BCAST_MASK = [0] * 32                     # broadcast lane 0 of each 32-block


def _build_nc():
    import concourse.mybir as mybir
    from concourse import bacc, tile
    from contextlib import ExitStack

    f32 = mybir.dt.float32
    bf16 = mybir.dt.bfloat16
    EXP = mybir.ActivationFunctionType.Exp

    nc = bacc.Bacc(
        "TRN2", target_bir_lowering=False, debug=False,
        enable_asserts=False, num_devices=B,
    )

    xt_d = nc.dram_tensor("xt", [P, 2, CT, 512], bf16, kind="ExternalInput")
    cos2_d = nc.dram_tensor("cos2", [P, N], bf16, kind="ExternalInput")
    sin2_d = nc.dram_tensor("sin2", [P, N], bf16, kind="ExternalInput")
    wqk_d = nc.dram_tensor("wqk", [TP, 2, P, CT * P], bf16, kind="ExternalInput")
    wv_d = nc.dram_tensor("wv", [P, 2, CT, 512], bf16, kind="ExternalInput")
    wp_d = nc.dram_tensor("wp", [P, CT, C], bf16, kind="ExternalInput")
    sel2_d = nc.dram_tensor("sel2", [2, P], bf16, kind="ExternalInput")
    bias_d = nc.dram_tensor("bias", [1, C], f32, kind="ExternalInput")
    out_d = nc.dram_tensor("out", [N, C], f32, kind="ExternalOutput")

    with tile.TileContext(nc) as tc, ExitStack() as ctx:
        const = ctx.enter_context(tc.tile_pool(name="const", bufs=1))
        vpool = ctx.enter_context(tc.tile_pool(name="vpool", bufs=1))
        otpool = ctx.enter_context(tc.tile_pool(name="otpool", bufs=1))
        qkpre = ctx.enter_context(tc.tile_pool(name="qkpre", bufs=2))
        qkfin = ctx.enter_context(tc.tile_pool(name="qkfin", bufs=4))
        expp = ctx.enter_context(tc.tile_pool(name="expp", bufs=3))
        cpsp = ctx.enter_context(tc.tile_pool(name="cpsp", bufs=4))
        nrmp = ctx.enter_context(tc.tile_pool(name="nrmp", bufs=2))
        ybp = ctx.enter_context(tc.tile_pool(name="ybp", bufs=3))
        sT_ps = ctx.enter_context(tc.tile_pool(name="sT_ps", bufs=2, space="PSUM"))
        av_ps = ctx.enter_context(tc.tile_pool(name="av_ps", bufs=2, space="PSUM"))
        mi_ps = ctx.enter_context(tc.tile_pool(name="mi_ps", bufs=2, space="PSUM"))

        # ---- input DMA, strict priority order (sync engine issues in
        # order); every transfer is contiguous per partition ----
        wqk = const.tile([P, TP, 2, CT, P], bf16)  # all q,k weight D-tiles
        for s in range(2):
            nc.sync.dma_start(
                wqk[:, 0, s].rearrange("p ct c -> p (ct c)"), wqk_d[0, s])
        cos2 = const.tile([P, N], bf16)
        sin2 = const.tile([P, N], bf16)
        nc.sync.dma_start(cos2[:, 0:512], cos2_d[:, 0:512])
        nc.sync.dma_start(sin2[:, 0:512], sin2_d[:, 0:512])
        xt = const.tile([P, 2, CT, 512], bf16)     # [p, ch, ct, n-within]
        nc.sync.dma_start(xt[:, 0], xt_d[:, 0])
        wv = const.tile([P, 2, CT, 512], bf16)     # [p, ch-of-dd, ct, dd]
        nc.sync.dma_start(wv[:, 0], wv_d[:, 0])
        nc.sync.dma_start(cos2[:, 512:N], cos2_d[:, 512:N])
        nc.sync.dma_start(sin2[:, 512:N], sin2_d[:, 512:N])
        nc.sync.dma_start(xt[:, 1], xt_d[:, 1])
        for t in range(1, TP):
            for s in range(2):
                nc.sync.dma_start(
                    wqk[:, t, s].rearrange("p ct c -> p (ct c)"), wqk_d[t, s])
        nc.sync.dma_start(wv[:, 1], wv_d[:, 1])
        wp = const.tile([P, CT, C], bf16)
        nc.sync.dma_start(wp[:], wp_d[:])
        bias_bc = const.tile([P, C], f32)
        nc.sync.dma_start(bias_bc[:1, :], bias_d[:])
        nc.gpsimd.partition_broadcast(bias_bc[:], bias_bc[:1, :])
        sel2 = const.tile([2, P], bf16)
        nc.sync.dma_start(sel2[:], sel2_d[:])

        def xts(ct, nt):
            """xT tile [128 c-local, 128 n] for (ct, n-tile nt)."""
            return xt[:, nt // 4, ct, (nt % 4) * P:(nt % 4 + 1) * P]

        # v storage: [128 j-local, NT j-tiles, H heads x (64 v + 1 ones col)]
        v_all = vpool.tile([P, NT, H * (HD + 1)], bf16)
        ones_c = const.tile([P, H], f32)
        nc.vector.memset(ones_c[:], 1.0)
        for jt in range(NT):
            nc.vector.tensor_copy(
                v_all[:, jt, :].rearrange("p (h c) -> p h c", c=HD + 1)[:, :, HD:],
                ones_c[:].rearrange("p (h o) -> p h o", o=1))
        # out.T accumulator: [128 c-local, CT c-tiles, 1024 n]
        outT = otpool.tile([P, CT, N], bf16)
        # normalization staging: reciprocal rows land on partitions
        # 0/32/64/96; the rest must be initialized for the shuffle's read
        st_bc = otpool.tile([P, N], f32)
        nc.vector.memset(st_bc[:], 1.0)

        qk_fin = {}
        spairs = {}
        v_ready = set()

        def prepare_pair(t):
            """Generator: qk D-tile matmuls + RoPE for pair t, yielding after
            each PE instruction. ch-chunked so the ch0 halves of q,k (and
            hence the first scores) never wait on ch1 inputs."""
            fins = [qkfin.tile([P, N], bf16, tag="fin", name=f"fin{s}")
                    for s in range(2)]
            qk_fin[t] = fins
            pres = [qkpre.tile([P, N], bf16, tag=f"pre{s}", name=f"pre{s}")
                    for s in range(2)]
            for ch in range(2):
                cs = slice(ch * 512, (ch + 1) * 512)
                for s in range(2):  # 0 = q, 1 = k
                    qps = mi_ps.tile([P, 512], f32, tag="mi", name="qps")
                    for ct in range(CT):
                        nc.tensor.matmul(
                            qps[:],
                            wqk[:, t, s, ct],
                            xt[:, ch, ct, :],
                            start=(ct == 0), stop=(ct == CT - 1),
                        )
                        yield
                    nc.vector.tensor_copy(pres[s][:, cs], qps[:])
                for s in range(2):
                    rot = qkpre.tile([P, 512], bf16, tag="rot", name="rot")
                    nc.vector.stream_shuffle(rot[:], pres[s][:, cs], SHUF_MASK)
                    tmp = qkpre.tile([P, 512], bf16, tag="tmp", name="tmp")
                    nc.vector.tensor_mul(tmp[:], pres[s][:, cs], cos2[:, cs])
                    nc.vector.tensor_mul(fins[s][:, cs], rot[:], sin2[:, cs])
                    nc.vector.tensor_add(fins[s][:, cs], fins[s][:, cs], tmp[:])
                    yield

        def v_group(nt, ch):
            """Generator: one v-projection group (8 accumulating matmuls +
            copy into the packed v_all layout)."""
            vps = mi_ps.tile([P, 512], f32, tag="mi", name="vps")
            for ct in range(CT):
                nc.tensor.matmul(
                    vps[:],
                    xts(ct, nt),
                    wv[:, ch, ct, :],
                    start=(ct == 0), stop=(ct == CT - 1),
                )
                yield
            nc.vector.tensor_copy(
                v_all[:, nt, :].rearrange(
                    "p (h c) -> p h c", c=HD + 1)[:, 8 * ch:8 * ch + 8, :HD],
                vps[:])
            v_ready.add((nt, ch))

        def chain(*gens):
            for g in gens:
                yield from g

        feeder = None

        def pull(k):
            nonlocal feeder
            if feeder is None:
                return
            for _ in range(k):
                if next(feeder, "done") == "done":
                    feeder = None
                    return

        def ensure_v(nt, vch):
            """Drain the feeder until v_group(nt, vch) has been emitted."""
            while (nt, vch) not in v_ready:
                assert feeder is not None, f"v_group({nt},{vch}) unreachable"
                pull(1)

        # ---- the flat attention pipeline ----
        pend = None   # (t, ch, jt, ex, avA, avB) awaiting its av matmuls

        def emit_pend():
            """Emit the trailing av matmuls; on a ch-pass close, also emit
            the epilogue copies and (on a pair close) the previous pair's
            normalization."""
            nonlocal pend
            if pend is None:
                return
            t, ch, jt, ex, avA, avB = pend
            pend = None
            hA, hB = 2 * t, 2 * t + 1
            ensure_v(jt, t // 4)
            nc.tensor.matmul(
                avA[:], v_all[:, jt, hA * (HD + 1):(hA + 1) * (HD + 1)],
                ex[:, 0:512], start=(jt == 0), stop=(jt == NT - 1))
            nc.tensor.matmul(
                avB[:], v_all[:, jt, hB * (HD + 1):(hB + 1) * (HD + 1)],
                ex[:, 512:1024], start=(jt == 0), stop=(jt == NT - 1))
            if jt == NT - 1:
                cs = slice(ch * 512, (ch + 1) * 512)
                spair = spairs[t]
                # rows 0:64 = unnormalized out.T; row 64 = softmax sums
                nc.vector.tensor_copy(outT[0:64, t, cs], avA[:HD, :])
                nc.vector.tensor_copy(outT[64:128, t, cs], avB[:HD, :])
                cpA = cpsp.tile([1, 512], f32, tag="cp", name="cpA")
                nc.vector.tensor_copy(cpA[:], avA[HD:HD + 1, :])
                nc.sync.dma_start(spair[0:1, cs], cpA[:])
                cpB = cpsp.tile([1, 512], f32, tag="cp", name="cpB")
                nc.vector.tensor_copy(cpB[:], avB[HD:HD + 1, :])
                nc.sync.dma_start(spair[1:2, cs], cpB[:])
                if ch == 1 and t >= 1:
                    normalize_gp(t - 1)

        def normalize_gp(t):
            """Per-pair softmax normalization of outT[:, t, :], emitted one
            pair late: DMA stages the reciprocal rows at partitions
            0/32/64/96, one stream_shuffle broadcasts within each 32-block,
            and the multiply runs on the otherwise-idle GPSIMD."""
            spair = spairs.pop(t)
            rcp2 = nrmp.tile([2, N], f32, tag="rcp2", name="rcp2")
            nc.vector.reciprocal_approx_fast(rcp2[:], spair[:])
            for q, r in ((0, 0), (32, 0), (64, 1), (96, 1)):
                nc.sync.dma_start(st_bc[q:q + 1, :], rcp2[r:r + 1, :])
            rb = nrmp.tile([P, N], f32, tag="rb", name="rb")
            nc.vector.stream_shuffle(rb[:], st_bc[:], BCAST_MASK)
            nc.gpsimd.tensor_mul(outT[:, t, :], outT[:, t, :], rb[:])

        def normalize_sel(t):
            """Pair-7 normalization: K=2 selection matmul + DVE multiplies —
            shortest latency chain for the tail (rb rides the freed sT tag)."""
            spair = spairs.pop(t)
            rcp2 = nrmp.tile([2, N], f32, tag="rcp2", name="rcp2")
            nc.vector.reciprocal_approx_fast(rcp2[:], spair[:])
            rcp16 = nrmp.tile([2, N], bf16, tag="rcp16", name="rcp16")
            nc.vector.tensor_copy(rcp16[:], rcp2[:])
            for ch in range(2):
                cs = slice(ch * 512, (ch + 1) * 512)
                rb = sT_ps.tile([P, 512], f32, tag="sT", name="rbps")
                nc.tensor.matmul(rb[:], sel2[:], rcp16[:, cs],
                                 start=True, stop=True)
                nc.vector.tensor_mul(outT[:, t, cs], outT[:, t, cs], rb[:])

        def attention(t, npull):
            qf, kf = qk_fin.pop(t)
            spairs[t] = nrmp.tile([2, N], f32, tag="spair", name="spair")
            for ch in range(2):
                cs = slice(ch * 512, (ch + 1) * 512)
                avA = av_ps.tile([HD + 1, 512], f32, tag="av", name="avA")
                avB = av_ps.tile([HD + 1, 512], f32, tag="av", name="avB")
                for jt in range(NT):
                    sT = sT_ps.tile([P, N], f32, tag="sT", name="sT")
                    js = slice(jt * P, (jt + 1) * P)
                    # two K=64 row-tiles, back-to-back -> concurrent on PE
                    nc.tensor.matmul(sT[:, 0:512], kf[0:64, js], qf[0:64, cs],
                                     start=True, stop=True)
                    nc.tensor.matmul(sT[:, 512:1024], kf[64:128, js],
                                     qf[64:128, cs], start=True, stop=True)
                    ex = expp.tile([P, N], bf16, tag="ex", name="ex")
                    nc.scalar.activation(ex[:], sT[:], EXP, scale=SCALE)
                    pull(npull)
                    emit_pend()
                    pend_set(t, ch, jt, ex, avA, avB)

        def pend_set(t, ch, jt, ex, avA, avB):
            nonlocal pend
            assert pend is None
            pend = (t, ch, jt, ex, avA, avB)

        def proj_group(nt, ch2, ct_hi):
            """Emit out-proj accumulation matmuls for cts [0, ct_hi)."""
            yps = mi_ps.tile([P, 512], f32, tag="mi", name="yps")
            es = slice(ch2 * 512, (ch2 + 1) * 512)
            ns = slice(nt * P, (nt + 1) * P)
            for ct in range(ct_hi):
                nc.tensor.matmul(
                    yps[:], outT[:, ct, ns], wp[:, ct, es],
                    start=(ct == 0), stop=False,
                )
            return yps

        def proj_close(nt, ch2, yps):
            es = slice(ch2 * 512, (ch2 + 1) * 512)
            ns = slice(nt * P, (nt + 1) * P)
            nc.tensor.matmul(
                yps[:], outT[:, CT - 1, ns], wp[:, CT - 1, es],
                start=False, stop=True,
            )
            yb = ybp.tile([P, 512], f32, tag="yb", name="yb")
            nc.vector.tensor_add(yb[:], yps[:], bias_bc[:, es])
            nc.sync.dma_start(out_d[ns, es], yb[:])

        # ---- prologue: pair-0 qk ch0-half only; the rest feeds pair 0 ----
        pp0 = prepare_pair(0)
        feeder = pp0
        pull(18)   # 16 ch0 matmuls + 2 RoPE-chunk yields

        # feeder work for each pair's attention window
        feeders = [
            chain(pp0, v_group(0, 0), v_group(1, 0), v_group(2, 0),
                  v_group(3, 0), v_group(4, 0), v_group(5, 0),
                  v_group(6, 0), v_group(7, 0), prepare_pair(1)),
            chain(v_group(0, 1), v_group(1, 1), v_group(2, 1), prepare_pair(2)),
            chain(v_group(3, 1), v_group(4, 1), v_group(5, 1), prepare_pair(3)),
            chain(v_group(6, 1), v_group(7, 1), prepare_pair(4)),
            prepare_pair(5),
            prepare_pair(6),
            prepare_pair(7),
            None,
        ]
        npulls = [6, 4, 4, 4, 3, 3, 3, 2]

        for t in range(TP):
            feeder = feeders[t] if feeders[t] is not None else feeder
            attention(t, npulls[t])
            pull(10 ** 6)
        emit_pend()            # flush (7, ch1, jt7) + epilogue + norm(6)

        # ---- out-proj: y = outT.T @ w_projT + bias ----
        # prefill two groups' ct 0..6 before pair-7 normalization is emitted
        # so the PE has work during the normalization chain
        open_groups = [(0, 0, proj_group(0, 0, CT - 1)),
                       (0, 1, proj_group(0, 1, CT - 1))]
        normalize_sel(TP - 1)
        for nt, ch2, yps in open_groups:
            proj_close(nt, ch2, yps)
        for nt in range(NT):
            for ch2 in range(2):
                if nt == 0:
                    continue
                proj_close(nt, ch2, proj_group(nt, ch2, CT - 1))

    nc.compile()
    return nc


def get_nc():
    if "nc" not in _CACHE:
        _CACHE["nc"] = _build_nc()
    return _CACHE["nc"]


def _host_inputs(x, xpos, w_qkv, w_proj, b_proj):
    """Host-side reshapes: transposes, RoPE tables, weight packing into the
    exact SBUF layouts (so device DMAs are contiguous)."""
    import ml_dtypes

    x = np.asarray(x, dtype=np.float32)
    xpos = np.asarray(xpos)
    w_qkv = np.asarray(w_qkv, dtype=np.float32)
    w_proj = np.asarray(w_proj, dtype=np.float32)
    b_proj = np.asarray(b_proj, dtype=np.float32).reshape(1, C)

    # xt[b] = [p, ch, ct, n-within]: xT[ct*128+p, ch*512+n]
    xT = x.transpose(0, 2, 1)                      # [B, C, N]
    xt = np.ascontiguousarray(
        xT.reshape(B, CT, P, 2, 512).transpose(0, 2, 3, 1, 4))

    # RoPE tables in [d, n] orientation, two head-copies stacked to 128 rows.
    inv_freq = (100.0 ** (-np.arange(16, dtype=np.float64) / 16.0))
    py = xpos[..., 0].astype(np.float64)  # [B, N]
    px = xpos[..., 1].astype(np.float64)
    angy = py[:, :, None] * inv_freq      # [B, N, 16]
    angx = px[:, :, None] * inv_freq
    cos64 = np.concatenate(
        [np.cos(angy), np.cos(angy), np.cos(angx), np.cos(angx)], axis=2)
    sin64 = np.concatenate(
        [-np.sin(angy), np.sin(angy), -np.sin(angx), np.sin(angx)], axis=2)
    cos2 = np.ascontiguousarray(
        np.tile(cos64, (1, 1, 2)).transpose(0, 2, 1)).astype(np.float32)
    sin2 = np.ascontiguousarray(
        np.tile(sin64, (1, 1, 2)).transpose(0, 2, 1)).astype(np.float32)

    # wqk[t, s] = [c-local partition, ct*P + d] — exact SBUF layout
    wqk = np.zeros((TP, 2, P, CT * P), dtype=np.float32)
    for t in range(TP):
        for s in range(2):
            rows = w_qkv[s * C + t * P:s * C + (t + 1) * P, :]  # [P(d), C]
            wqk[t, s] = rows.reshape(P, CT, P).transpose(2, 1, 0).reshape(
                P, CT * P)
    # wv = [c-local partition, ch-of-dd, ct, dd-within]
    wv = np.ascontiguousarray(
        w_qkv[2 * C:3 * C, :].T.reshape(CT, P, 2, 512).transpose(1, 2, 0, 3))
    # wp = [c-local partition, ct, e]
    wp = np.ascontiguousarray(w_proj.T.reshape(CT, P, C).transpose(1, 0, 2))

    sel2 = np.zeros((2, P), dtype=np.float32)
    sel2[0, :HD] = 1.0
    sel2[1, HD:] = 1.0

    def mcast(a):
        return np.ascontiguousarray(a).astype(ml_dtypes.bfloat16)

    shared = dict(wqk=mcast(wqk), wv=mcast(wv), wp=mcast(wp),
                  sel2=mcast(sel2), bias=b_proj)
    in_maps = []
    for b in range(B):
        m = dict(shared)
        m["xt"] = mcast(xt[b])
        m["cos2"] = mcast(cos2[b])
        m["sin2"] = mcast(sin2[b])
        in_maps.append(m)
    return in_maps


def kernel(x, xpos, w_qkv, w_proj, b_proj):
    from concourse import bass_utils

    nc = get_nc()
    in_maps = _host_inputs(x, xpos, w_qkv, w_proj, b_proj)
    res = bass_utils.run_bass_kernel_spmd(
        nc, in_maps, core_ids=list(range(B)),
        trace=bool(int(os.environ.get("BASS_ATTN_TRACE", "0"))),
    )
    out = np.stack([res.results[b]["out"] for b in range(B)], axis=0)
    _CACHE["last_results"] = res
    return out
